# revision 1
# baseline (speedup 1.0000x reference)
"""Bass/Trainium2 kernel for nn_BiasedAxialAttention (triangle attention, is_row).

Self-contained: builds a Bass/Tile SPMD program, shards inputs over 8
NeuronCores host-side, runs via run_bass_kernel_spmd, gathers the output.

Sharding: the tied contraction axis n (pair columns) is split 8 ways.
Each core computes a partial [H, L, L] logit tensor (fp16), the partials
are AllReduced per 128-row chunk (bias@Wb folded into the reduction),
and each core then produces its own 48 rows of the final output.

v2 layout/dtype strategy:
  - pair slices arrive as fp16, partition-major [128, R, NIC, 128] so one
    DMA loads a whole 8-slab group with contiguous 6KB partition lines.
  - V stays in SBUF (no DRAM round trip).
  - logits AllReduce in fp16, split into 3 per-ic collectives so softmax
    of chunk 0 overlaps the reduction of chunks 1-2.
  - out-projection emits [D, i] (transposed); host untransposes. fp16 out.
"""

import math
from contextlib import ExitStack

import numpy as np

import concourse.bacc as bacc
import concourse.bass as bass
import concourse.tile as tile
from concourse import mybir
from concourse.bass_utils import run_bass_kernel_spmd

F32 = mybir.dt.float32
F16 = mybir.dt.float16

D = 128          # pair feature dim (= D_PAIR = D_BIAS)
H = 4            # heads
DH = 32          # head dim
NCORES = 8
L_FULL = 384

GS = 8           # slabs per stats group (one sqrt+recip per group)

# engine assignment knobs (tuned from trace)
NORM1_ENGINE = "vector"   # pass-1 LN normalize
NORM2_ENGINE = "vector"   # pass-2 LN normalize (scalar queue must stay clear for exps)
VEVAC_ENGINE = "vector"   # V PSUM->SBUF evac
SLABEVAC_ENGINE = "scalar"
QKEVAC_ENGINE = "scalar"
UTEVAC_ENGINE = "scalar"


def build_program(L, NC, *, has_bq=False, has_bk=False, has_bv=False,
                  has_bo=False, debug=False):
    """Emit the SPMD program (identical for every core)."""
    assert L % 128 == 0
    NIC = L // 128          # number of 128-row chunks of L
    R = L // NC             # rows owned by each core
    assert R % 2 == 0
    nc = bacc.Bacc("TRN2", target_bir_lowering=False, debug=debug,
                   num_devices=NC)

    # ---- kernel I/O (per-core slices, host-prepared) ----
    pc16 = nc.dram_tensor("pc16", [128, R, NIC, D], F16,
                          kind="ExternalInput").ap()
    pr16 = nc.dram_tensor("pr16", [128, R, NIC, D], F16,
                          kind="ExternalInput").ap()
    bp16 = nc.dram_tensor("bp16", [H, NIC, 128, L], F16,
                          kind="ExternalInput").ap()
    wcols = nc.dram_tensor("wcols", [D, 4], F32, kind="ExternalInput").ap()
    w16 = nc.dram_tensor("w16", [6, D, D], F16, kind="ExternalInput").ap()
    bro = nc.dram_tensor("bro", [2, 1, NIC * D], F32,
                         kind="ExternalInput").ap()
    out = nc.dram_tensor("out", [R, D, L], F16, kind="ExternalOutput").ap()

    def eng(name):
        return {"gpsimd": nc.gpsimd, "vector": nc.vector,
                "scalar": nc.scalar}[name]

    def copy_op(name):
        if name == "scalar":
            return nc.scalar.copy
        return eng(name).tensor_copy

    with tile.TileContext(nc) as tc, ExitStack() as ctx:
        consts = ctx.enter_context(tc.tile_pool(name="consts", bufs=1))
        persist = ctx.enter_context(tc.tile_pool(name="persist", bufs=1))
        rot = ctx.enter_context(tc.tile_pool(name="rot", bufs=3))
        dram = ctx.enter_context(tc.tile_pool(name="dram", bufs=1, space="DRAM"))

        # ---- constants ----
        w16_sb = consts.tile([128, 6, D], F16, name="w16_sb", tag="w16_sb")
        nc.sync.dma_start(out=w16_sb, in_=w16.rearrange("a p d -> p a d"))
        wo_sb = w16_sb[:, 0, :]
        id16_sb = w16_sb[:, 1, :]
        wq_sb = w16_sb[:, 2, :]
        wk_sb = w16_sb[:, 3, :]
        wv_sb = w16_sb[:, 4, :]
        wg_sb = w16_sb[:, 5, :]
        wcols_sb = consts.tile([128, 4], F32, name="wcols_sb", tag="wcols_sb")
        nc.sync.dma_start(out=wcols_sb, in_=wcols)
        bg_col = wcols_sb[:, 0:1]
        eps_col = consts.tile([128, 1], F32, name="eps_col", tag="eps_col")
        nc.vector.memset(eps_col, 1e-5)
        if has_bv or has_bo:
            # bro[0] = bv tiled; bro[1] = bo-as-row (for the transposed proj)
            bro_t = consts.tile([2, 1, NIC * D], F32, name="bro_t", tag="bro_t")
            nc.sync.dma_start(out=bro_t, in_=bro)
            ones_t = consts.tile([1, L], F16, name="ones_t", tag="ones_t")
            nc.vector.memset(ones_t, 1.0)

        # persistent V (all own rows) and softmax row sums
        v_all = persist.tile([128, R, NIC, 128], F16, name="v_all",
                             tag="v_all")
        s_buf = persist.tile([128, H * NIC], F32, name="s_buf", tag="s_buf")
        rcp_buf = persist.tile([128, H * NIC], F32, name="rcp_buf",
                               tag="rcp_buf")

        # AllReduce bounce buffers (fp16), one pair per i-chunk
        arin = [dram.tile([H, 128, L], F16, name=f"arin{ic}", tag=f"arin{ic}")
                for ic in range(NIC)]
        arout = [dram.tile([H, 128, L], F16, name=f"arout{ic}",
                           tag=f"arout{ic}", addr_space="Shared")
                 for ic in range(NIC)]

        def ln_stage1(src_ap, g0, gn, tag):
            """One DMA for the whole group + per-slab stats."""
            xin = rot.tile([128, GS, NIC, D], F16, name=f"xin{tag}{g0}",
                           tag="xin", bufs=4)
            nc.sync.dma_start(out=xin[:, :gn], in_=src_ap[:, g0:g0 + gn])
            mvg = rot.tile([128, GS, NIC, 2], F32, name=f"mvg{tag}{g0}",
                           tag="mvg", bufs=3)
            rsg = rot.tile([128, GS, NIC], F32, name=f"rsg{tag}{g0}",
                           tag="rsg", bufs=3)
            nmg = rot.tile([128, GS, NIC], F32, name=f"nmg{tag}{g0}",
                           tag="nmg", bufs=3)
            for g in range(gn):
                for ic in range(NIC):
                    st = rot.tile([128, 6], F32, name=f"st{tag}{g0}_{g}{ic}",
                                  tag="st", bufs=8)
                    nc.vector.bn_stats(out=st, in_=xin[:, g, ic, :])
                    nc.vector.bn_aggr(out=mvg[:, g, ic, :], in_=st)
            # rstd = 1/sqrt(var + eps), one op pair for the whole group
            nc.scalar.activation(out=rsg, in_=mvg[:, :, :, 1],
                                 func=mybir.ActivationFunctionType.Sqrt,
                                 bias=eps_col, scale=1.0)
            nc.vector.reciprocal(out=rsg, in_=rsg)
            # nmg = -mu * rstd (bias column for activation-style normalize)
            nc.vector.scalar_tensor_tensor(
                out=nmg, in0=mvg[:, :, :, 0], scalar=-1.0, in1=rsg,
                op0=mybir.AluOpType.mult, op1=mybir.AluOpType.mult)
            return xin, mvg, rsg, nmg

        def ln_norm(xin, g, mvg, rsg, nmg, xr, norm_name):
            if norm_name == "mixed":
                for ic in range(NIC):
                    if ic == 2:
                        nc.scalar.activation(
                            out=xr[:, ic, :], in_=xin[:, g, ic, :],
                            func=mybir.ActivationFunctionType.Identity,
                            bias=nmg[:, g, ic:ic + 1],
                            scale=rsg[:, g, ic:ic + 1])
                    else:
                        nc.vector.tensor_scalar(
                            out=xr[:, ic, :], in0=xin[:, g, ic, :],
                            scalar1=mvg[:, g, ic, 0:1],
                            scalar2=rsg[:, g, ic:ic + 1],
                            op0=mybir.AluOpType.subtract,
                            op1=mybir.AluOpType.mult)
                return
            if norm_name == "scalar":
                for ic in range(NIC):
                    nc.scalar.activation(
                        out=xr[:, ic, :], in_=xin[:, g, ic, :],
                        func=mybir.ActivationFunctionType.Identity,
                        bias=nmg[:, g, ic:ic + 1],
                        scale=rsg[:, g, ic:ic + 1])
            else:
                for ic in range(NIC):
                    eng(norm_name).tensor_scalar(
                        out=xr[:, ic, :], in0=xin[:, g, ic, :],
                        scalar1=mvg[:, g, ic, 0:1],
                        scalar2=rsg[:, g, ic:ic + 1],
                        op0=mybir.AluOpType.subtract,
                        op1=mybir.AluOpType.mult)

        # =================== pass 1: Q/K/V + logits ===================
        with tc.tile_pool(name="qk_pool", bufs=1) as qk_pool, \
             tc.tile_pool(name="ln_ps", bufs=1, space="PSUM") as ln_ps, \
             tc.tile_pool(name="proj_ps", bufs=3, space="PSUM") as proj_ps, \
             tc.tile_pool(name="z_ps", bufs=1, space="PSUM") as z_ps:
            qt, kt = [], []

            def vt_proj(x, slab):
                vp = proj_ps.tile([128, L], F32, name=f"vp{x}", tag="proj")
                for jc in range(NIC):
                    nc.tensor.matmul(out=vp[:, jc * 128:(jc + 1) * 128],
                                     lhsT=slab[:, jc, :], rhs=wv_sb,
                                     start=True, stop=(not has_bv))
                    if has_bv:
                        nc.tensor.matmul(
                            out=vp[:, jc * 128:(jc + 1) * 128],
                            lhsT=ones_t[:, 0:1],
                            rhs=bro_t[0, :, jc * D:(jc + 1) * D],
                            start=False, stop=True)
                copy_op("scalar")(
                    out=v_all[:, x], in_=vp.rearrange("p (a b) -> p a b",
                                                      a=NIC))

            def qk_proj(x, slab):
                qp = proj_ps.tile([128, L], F32, name=f"qp{x}", tag="proj")
                nc.tensor.matmul(out=qp, lhsT=wq_sb, rhs=slab,
                                 start=True, stop=True)
                q_sb = qk_pool.tile([128, L], F16, name=f"q{x}", tag=f"q{x}")
                if has_bq:
                    nc.scalar.activation(
                        out=q_sb, in_=qp,
                        func=mybir.ActivationFunctionType.Identity,
                        bias=wcols_sb[:, 1:2], scale=1.0)
                else:
                    copy_op(QKEVAC_ENGINE)(out=q_sb, in_=qp)
                qt.append(q_sb)
                kp = proj_ps.tile([128, L], F32, name=f"kp{x}", tag="proj")
                nc.tensor.matmul(out=kp, lhsT=wk_sb, rhs=slab,
                                 start=True, stop=True)
                k_sb = qk_pool.tile([128, L], F16, name=f"k{x}", tag=f"k{x}")
                if has_bk:
                    nc.scalar.activation(
                        out=k_sb, in_=kp,
                        func=mybir.ActivationFunctionType.Identity,
                        bias=wcols_sb[:, 2:3], scale=1.0)
                else:
                    copy_op(QKEVAC_ENGINE)(out=k_sb, in_=kp)
                kt.append(k_sb)

            for g0 in range(0, R, GS):
                gn = min(GS, R - g0)
                xin, mvg, rsg, nmg = ln_stage1(pc16, g0, gn, "c")
                for p0 in range(0, gn, 2):
                    x0 = g0 + p0
                    lps2 = ln_ps.tile([128, 2, NIC, 128], F16,
                                      name=f"lps{x0}", tag="lps")
                    slab2 = rot.tile([128, 2, NIC, 128], F16,
                                     name=f"xh{x0}", tag="xh", bufs=6)
                    for par in range(2):
                        g = p0 + par
                        xr = rot.tile([128, NIC, D], F16,
                                      name=f"xr{g0 + g}", tag="xr", bufs=8)
                        ln_norm(xin, g, mvg, rsg, nmg, xr, NORM1_ENGINE)
                        for ic in range(NIC):
                            nc.tensor.transpose(out=lps2[:, par, ic, :],
                                                in_=xr[:, ic, :],
                                                identity=id16_sb)
                    copy_op(SLABEVAC_ENGINE)(out=slab2, in_=lps2)
                    for par in range(2):
                        x = x0 + par
                        qk_proj(x, slab2[:, par])
                        vt_proj(x, slab2[:, par])

            # partial logits Z[h][ic] = sum_x Q_x^T K_x (K=32, row-tiled)
            for ic in range(NIC):
                zts = [z_ps.tile([128, L], F32, name=f"z{ic}_{h}",
                                 tag=f"z{h}") for h in range(H)]
                for x in range(R):
                    for h in range(H):
                        nc.tensor.matmul(
                            out=zts[h],
                            lhsT=qt[x][32 * h:32 * h + 32,
                                       ic * 128:(ic + 1) * 128],
                            rhs=kt[x][32 * h:32 * h + 32, :],
                            start=(x == 0), stop=(x == R - 1),
                            tile_position=(32 * h, 0))
                for h in range(H):
                    bpt = rot.tile([128, L], F16, name=f"bpt{ic}_{h}",
                                   tag="bpt", bufs=4)
                    nc.gpsimd.dma_start(out=bpt, in_=bp16[h, ic])
                    zst = rot.tile([128, L], F16, name=f"zst{ic}_{h}",
                                   tag="zst", bufs=4)
                    nc.vector.tensor_add(out=zst, in0=zts[h], in1=bpt)
                    nc.sync.dma_start(out=arin[ic][h], in_=zst)
                # per-chunk AllReduce: overlaps later chunks + softmax
                nc.gpsimd.collective_compute(
                    "AllReduce", mybir.AluOpType.add,
                    replica_groups=[list(range(NC))],
                    ins=[arin[ic].opt()], outs=[arout[ic].opt()])

        # =================== pass 2: gate ===================
        g_pool = ctx.enter_context(tc.tile_pool(name="g_pool", bufs=1))
        gt = []
        with tc.tile_pool(name="ln2_ps", bufs=2, space="PSUM") as ln2_ps, \
             tc.tile_pool(name="gp_ps", bufs=3, space="PSUM") as gp_ps:

            for g0 in range(0, R, GS):
                gn = min(GS, R - g0)
                xin, mvg, rsg, nmg = ln_stage1(pr16, g0, gn, "r")
                n2_eng = "vector" if (g0 // GS) % 2 == 0 else "scalar"
                for p0 in range(0, gn, 2):
                    lps2 = ln2_ps.tile([128, 2, NIC, 128], F16,
                                       name=f"lpr{g0 + p0}", tag="lpr")
                    slab2 = rot.tile([128, 2, NIC, 128], F16,
                                     name=f"rh{g0 + p0}", tag="rh", bufs=4)
                    for par in range(2):
                        g = p0 + par
                        xr = rot.tile([128, NIC, D], F16,
                                      name=f"xrr{g0 + g}", tag="xrr", bufs=6)
                        ln_norm(xin, g, mvg, rsg, nmg, xr, n2_eng)
                        for ic in range(NIC):
                            nc.tensor.transpose(out=lps2[:, par, ic, :],
                                                in_=xr[:, ic, :],
                                                identity=id16_sb)
                    nc.vector.tensor_copy(out=slab2, in_=lps2)
                    for par in range(2):
                        x = g0 + p0 + par
                        gp = gp_ps.tile([128, L], F32, name=f"gp{x}",
                                        tag="gp")
                        nc.tensor.matmul(out=gp, lhsT=wg_sb,
                                         rhs=slab2[:, par],
                                         start=True, stop=True)
                        g_sb = g_pool.tile([128, L], F16, name=f"g{x}",
                                           tag=f"g{x}")
                        nc.scalar.activation(
                            out=g_sb, in_=gp,
                            func=mybir.ActivationFunctionType.Sigmoid,
                            bias=bg_col, scale=1.0)
                        gt.append(g_sb)

        # =================== post-AllReduce ===================
        at_pool = ctx.enter_context(tc.tile_pool(name="at_pool", bufs=1))
        with tc.tile_pool(name="at_ps", bufs=2, space="PSUM") as at_ps:

            # softmax over j (in [i, j] layout) + transpose A -> [j, i]
            # (at_ps is only 2 banks so this overlaps the gate's PSUM pools)
            at_sb = [[None] * NIC for _ in range(H)]
            for h in range(H):
                e2s = []
                for ic in range(NIC):
                    idx = h * NIC + ic
                    zsum = rot.tile([128, L], F16, name=f"zs{h}_{ic}",
                                    tag="zsum", bufs=4)
                    nc.sync.dma_start(out=zsum, in_=arout[ic][h])
                    e_t = rot.tile([128, L], F16, name=f"e{h}_{ic}",
                                   tag="e", bufs=4)
                    nc.scalar.activation(
                        out=e_t, in_=zsum,
                        func=mybir.ActivationFunctionType.Exp,
                        accum_out=s_buf[:, idx:idx + 1])
                    nc.vector.reciprocal(out=rcp_buf[:, idx:idx + 1],
                                         in_=s_buf[:, idx:idx + 1])
                    e2_t = rot.tile([128, L], F16, name=f"e2{h}_{ic}",
                                    tag="e2", bufs=8)
                    nc.vector.tensor_scalar_mul(
                        out=e2_t, in0=e_t,
                        scalar1=rcp_buf[:, idx:idx + 1])
                    e2s.append(e2_t)
                for jc in range(NIC):
                    atp = at_ps.tile([128, NIC, 128], F16,
                                     name=f"atp{h}_{jc}", tag="atp")
                    for ic in range(NIC):
                        nc.tensor.transpose(
                            out=atp[:, ic, :],
                            in_=e2s[ic][:, jc * 128:(jc + 1) * 128],
                            identity=id16_sb)
                    a_sb = at_pool.tile([128, NIC, 128], F16,
                                        name=f"at{h}_{jc}",
                                        tag=f"at{h}_{jc}")
                    nc.vector.tensor_copy(out=a_sb, in_=atp)
                    at_sb[h][jc] = a_sb

        # AV (col-tiled over heads) + gate + out-proj + store
        with tc.tile_pool(name="o_ps", bufs=3, space="PSUM") as o_ps, \
             tc.tile_pool(name="u_ps", bufs=3, space="PSUM") as u_ps:
            for x in range(R):
                ops_ = o_ps.tile([128, L], F32, name=f"o{x}", tag="o")
                for jc in range(NIC):
                    for h in range(H):
                        nc.tensor.matmul(
                            out=ops_[32 * h:32 * h + 32, :],
                            lhsT=v_all[:, x, jc, 32 * h:32 * h + 32],
                            rhs=at_sb[h][jc],
                            start=(jc == 0), stop=(jc == NIC - 1),
                            tile_position=(0, 32 * h))
                go = rot.tile([128, L], F16, name=f"go{x}", tag="go",
                              bufs=4)
                nc.vector.tensor_mul(out=go, in0=ops_, in1=gt[x])
                # out-proj, transposed: ups = Wo^T @ go = out[x]^T [D, i]
                ups = u_ps.tile([128, L], F32, name=f"u{x}", tag="u")
                nc.tensor.matmul(out=ups, lhsT=wo_sb, rhs=go,
                                 start=True, stop=(not has_bo))
                if has_bo:
                    nc.tensor.matmul(out=ups, lhsT=bro_t[1, :, 0:D],
                                     rhs=ones_t, start=False, stop=True)
                ut = rot.tile([128, L], F16, name=f"ut{x}", tag="ut",
                              bufs=4)
                copy_op(UTEVAC_ENGINE)(out=ut, in_=ups)
                nc.gpsimd.dma_start(out=out[x], in_=ut)

    nc.compile()
    return nc


def prep_inputs(pair, bias, ln_g, ln_b, Wq, Wk, Wv, Wb, Wg, bg, Wo, bo,
                L, NC):
    f32 = np.float32
    f16 = np.float16
    p2 = np.asarray(pair, f32)[0]
    R = L // NC
    NIC = L // 128
    ln_g = np.asarray(ln_g, f32)
    ln_b = np.asarray(ln_b, f32)
    Wq = np.asarray(Wq, f32)
    Wk = np.asarray(Wk, f32)
    Wv = np.asarray(Wv, f32)
    Wg = np.asarray(Wg, f32)
    Wo = np.asarray(Wo, f32)
    sc_q = 1.0 / math.sqrt(DH)
    sc_k = 1.0 / math.sqrt(L)
    Wq_eff = ln_g[:, None] * Wq * sc_q
    Wk_eff = ln_g[:, None] * Wk * sc_k
    Wv_eff = ln_g[:, None] * Wv
    Wg_eff = ln_g[:, None] * Wg
    bq = (ln_b @ Wq) * sc_q
    bk = (ln_b @ Wk) * sc_k
    bv = ln_b @ Wv
    bgE = ln_b @ Wg + np.asarray(bg, f32)
    bo = np.asarray(bo, f32)
    BP = np.einsum("ijk,kh->hij", np.asarray(bias, f32)[0],
                   np.asarray(Wb, f32)).astype(f32)
    wcols = np.stack([bgE, bq, bk, bv], 1).astype(f32)
    w16 = np.stack([Wo, np.eye(D, dtype=f32), Wq_eff, Wk_eff,
                    Wv_eff, Wg_eff], 0).astype(f16)
    bro = np.stack([np.tile(bv, NIC), np.tile(bo, NIC)], 0)
    bro = bro.reshape(2, 1, NIC * D).astype(f32)
    pcT = p2.transpose(1, 0, 2)   # [n, i, D] (LN rows for Q/K/V)
    flags = dict(has_bq=bool(np.any(bq != 0)), has_bk=bool(np.any(bk != 0)),
                 has_bv=bool(np.any(bv != 0)), has_bo=bool(np.any(bo != 0)))
    in_maps = []
    for c in range(NC):
        sl = slice(c * R, (c + 1) * R)
        bp_c = np.zeros((H, L, L), f32)
        bp_c[:, sl, :] = BP[:, sl, :]
        # partition-major swizzle: [p, x, a, d] = src[x, a*128+p, d]
        pc_sw = pcT[sl].reshape(R, NIC, 128, D).transpose(2, 0, 1, 3)
        pr_sw = p2[sl].reshape(R, NIC, 128, D).transpose(2, 0, 1, 3)
        in_maps.append({
            "pc16": np.ascontiguousarray(pc_sw).astype(f16),
            "pr16": np.ascontiguousarray(pr_sw).astype(f16),
            "bp16": np.ascontiguousarray(
                bp_c.reshape(H, NIC, 128, L)).astype(f16),
            "wcols": wcols,
            "w16": w16,
            "bro": bro,
        })
    return in_maps, flags


def gather_output(results, L, NC):
    # out is [R, D, L] fp16 per core (transposed); untranspose + upcast
    parts = [np.asarray(r["out"], np.float32).transpose(0, 2, 1)
             for r in results]
    full = np.concatenate(parts, axis=0)
    return np.ascontiguousarray(full.reshape(1, L, L, D))


_CACHED = {}
_WARM = set()
TRACE = False          # set True (e.g. from test.py) to capture an NTFF trace
LAST_RESULT = None     # BassKernelResults of the most recent kernel() call


def kernel(**inputs):
    global LAST_RESULT
    L = int(np.asarray(inputs["pair"]).shape[1])
    NC = NCORES
    in_maps, flags = prep_inputs(
        inputs["pair"], inputs["bias"], inputs["ln_g"], inputs["ln_b"],
        inputs["Wq"], inputs["Wk"], inputs["Wv"], inputs["Wb"], inputs["Wg"],
        inputs["bg"], inputs["Wo"], inputs["bo"], L, NC)
    key = (L, NC, tuple(sorted(flags.items())))
    if key not in _CACHED:
        _CACHED[key] = build_program(L, NC, **flags)
    nc = _CACHED[key]
    if key not in _WARM:
        # first executions after load pay cold-start costs (collective
        # first-call staging, cold DMA rings/HAM); warm up untraced first
        for _ in range(3):
            run_bass_kernel_spmd(nc, in_maps, core_ids=list(range(NC)),
                                 trace=False)
        _WARM.add(key)
    res = run_bass_kernel_spmd(nc, in_maps, core_ids=list(range(NC)),
                               trace=TRACE)
    LAST_RESULT = res
    return gather_output(res.results, L, NC)

